# revision 37
# baseline (speedup 1.0000x reference)
"""DeepSeek-style MoE block (SwiGLU experts, top-k routing) on 8 Trainium2 cores.

Expert-parallel sharding: each of the 8 cores owns E/8 = 2 experts and receives
only the tokens routed to those experts (host-side dispatch). The device kernel
computes, per expert e with gathered/padded tokens XT [D, TG] (transposed):

    GT = W0e @ X^T          (PSUM, bf16 matmuls, DFF on partitions)
    UT = W1e @ X^T
    HT = silu(s0*GT) * UT   (SBUF, [DFF, TG])
    Y  = (HT)^T @ W2e^T     (tokens on partitions)
    Yout = coef[token] * Y  where coef = s1*s2*cw  (cw = summed routing weights)

The host then scatter-adds each expert's Yout rows into the dense [T, D] output
(the unshard/combine step for expert-parallel sharding).

Perf notes (from NTFF traces):
 - all matmul operands (xt, w01, w2, ht) are bf16: 1 col/cycle warm PE rate
   with half the fp32 DMA traffic, and bf16 enables the compiler's fast
   weight load (FWL) so LDWEIGHTS hides under the matmul stream (fp32
   LDWEIGHTS at ~198 ns/load nearly serialized with the MMs). PSUM
   accumulation and the activation path stay fp32.
 - Experts are sorted by routed-token count and paired big+small onto
   cores: slot 0 pads to the max big count, slot 1 to the max small count
   (ceil-8), trimming ~10% of the padded matmul columns vs uniform 384.
 - TRN2 has two physical HWDGE rings (SP + Activation). All large DMAs
   alternate between nc.sync and nc.scalar so the two rings run in parallel.
 - DMA pacing is everything: the 16 SDMA engines round-robin over in-flight
   transfers at packet granularity, so completions are fair-shared, not
   issue-ordered. w01 streams as 512 KB 4-k-tile chunks (big enough that the
   first chunk isn't starved by its siblings, small enough not to delay the
   first matmul), and each expert's w2 + the next expert's xt/fg0-w01 are
   issued behind the full w01 stream on the ring FIFO so they cannot steal
   HBM bandwidth from the critical phase-1 feed.
 - Split PSUM pools (6 banks phase 1 + 2 banks phase 2) so the phases never
   chain on each other's bank releases at expert/phase boundaries.
"""

import os
import numpy as np

T, D, DFF, E, TOPK = 1024, 2048, 1024, 16, 6
NCORES, P = 8, 128
EPC = E // NCORES  # experts per core

# Set by kernel() after each run: BassKernelResults (exec_time_ns when traced).
LAST_RESULT = None

_PROGRAM_CACHE = {}

XC = 4   # xt k-tiles per DMA chunk
KP = 8   # w01 k-tiles per DMA chunk


def _mwidths(TG):
    """Token-tile widths for phase 2 (tokens become PSUM partitions)."""
    out = []
    left = TG
    while left > 0:
        out.append(min(P, left))
        left -= P
    return out


def _build_program(TGs, d=D, dff=DFF, use_silu=True):
    """Build + compile the SPMD single-core Bass program.

    TGs: per-slot padded token counts (multiples of 8, <= 512).
    use_silu=False decomposes silu into sigmoid+mul (CoreSim lacks Silu).
    """
    import concourse.bacc as bacc
    import concourse.mybir as mybir
    import concourse.tile as tile

    f32 = mybir.dt.float32
    bf16 = mybir.dt.bfloat16
    Silu = mybir.ActivationFunctionType.Silu

    KD = d // P        # k-tiles over D (contraction of W0/W1 matmuls)
    KF = dff // P      # k-tiles over DFF (contraction of W2 matmul)
    DSW = min(512, d)  # output D slice width
    NDS = d // DSW     # output D slices
    FG = 2             # DFF f-tiles per PSUM group (2G + 2U = 4 banks/group)
    FGP = FG * P
    NFG = KF // FG
    NXC = KD // XC
    KD2 = KD // KP
    NTmax = max(len(_mwidths(TG)) for TG in TGs)

    assert all(TG <= 512 for TG in TGs)

    nc = bacc.Bacc("TRN2", target_bir_lowering=False, debug=False)

    xt_ds = [nc.dram_tensor(f"xt{e}", [NXC, P, XC, TGs[e]], bf16,
                            kind="ExternalInput").ap() for e in range(EPC)]
    w01_d = nc.dram_tensor("w01", [EPC, NFG, KD2, P, KP, 2, FGP], bf16,
                           kind="ExternalInput").ap()
    w2t_d = nc.dram_tensor("w2t", [EPC, NDS, P, KF, DSW], bf16,
                           kind="ExternalInput").ap()
    sc_d = nc.dram_tensor("scv", [EPC, P, 1 + NTmax], f32,
                          kind="ExternalInput").ap()
    # coef replicated across partitions: phase 2 scales PSUM columns (tokens)
    # during the PSUM->SBUF cast, so the per-token combine weight needs to be
    # available on every output partition.
    cp_ds = [nc.dram_tensor(f"coefp{e}", [P, TGs[e]], f32,
                            kind="ExternalInput").ap() for e in range(EPC)]
    # y layout [KD, P, TG]: phase-2 output has D on partitions, tokens free
    y_ds = [nc.dram_tensor(f"y{e}", [KD, P, TGs[e]], bf16,
                           kind="ExternalOutput").ap() for e in range(EPC)]

    with tile.TileContext(nc) as tc:
        # Alternate large DMAs across the two physical HWDGE rings.
        rings = [nc.sync, nc.scalar]
        ring_state = [0]

        def ring():
            ring_state[0] ^= 1
            return rings[ring_state[0]]

        with (
            tc.tile_pool(name="xt", bufs=1) as xt_pool,
            tc.tile_pool(name="w01", bufs=3) as w01_pool,
            tc.tile_pool(name="w2", bufs=4) as w2_pool,
            tc.tile_pool(name="ht", bufs=1) as ht_pool,
            tc.tile_pool(name="act", bufs=6) as act_pool,
            tc.tile_pool(name="out", bufs=8) as out_pool,
            tc.tile_pool(name="sc", bufs=2) as sc_pool,
            tc.tile_pool(name="pgu", bufs=6, space="PSUM") as pgu_pool,
            tc.tile_pool(name="py", bufs=2, space="PSUM") as py_pool,
        ):
            # Tiles created by the previous expert's prefetch block (the ring
            # FIFO paces their descriptor-gen behind the in-order w01 stream,
            # so the prefetch costs no early bandwidth).
            xts = {}        # e -> xt tile (per-slot tag: bufs=1 suffices)
            pre01 = {}      # (e, kb) -> fg0 w01 chunk tile

            def get_xt(e):
                if e not in xts:
                    xts[e] = xt_pool.tile([P, KD, TGs[e]], bf16, tag=f"xt{e}",
                                          name=f"xt_{e}")
                return xts[e]

            for e in range(EPC):
                TG = TGs[e]

                # --- inputs for this expert ---
                xt = get_xt(e)
                scv = sc_pool.tile([P, 1 + NTmax], f32, tag="scv")
                nc.gpsimd.dma_start(scv[:], sc_d[e])
                s0_sb = scv[:, 0:1]
                coefp = sc_pool.tile([P, TG], f32, tag=f"cp{e}",
                                     name=f"coefp_{e}")
                nc.gpsimd.dma_start(coefp[:], cp_ds[e])

                # --- phase 1: HT = silu(s0 * W0 xT) * (W1 xT), [DFF, TG] ---
                # w2 chunks are issued at f-group boundaries (fg>=1): the ring
                # FIFO paces them behind that f-group's w01 stream, spreading
                # the ~4 MB w2 prefetch across phase 1 instead of cramming it
                # into the (now short) phase-2 window — while keeping it out
                # of the cold-start fg0 window.
                w2bs = []

                def issue_w2():
                    dsi = len(w2bs)
                    w2b = w2_pool.tile([P, KF, DSW], bf16, tag="w2b",
                                       name=f"w2b_{e}_{dsi}")
                    ring().dma_start(w2b[:], w2t_d[e, dsi])
                    w2bs.append(w2b)

                ht = ht_pool.tile([P, KF, TG], bf16, tag=f"ht{e}")
                for fg in range(NFG):
                    if fg >= 1 and len(w2bs) < NDS:
                        issue_w2()
                    psG = [pgu_pool.tile([P, 512], f32, tag="pgu",
                                         name=f"psG_{e}_{fg}_{j}")
                           for j in range(FG)]
                    psU = [pgu_pool.tile([P, 512], f32, tag="pgu",
                                         name=f"psU_{e}_{fg}_{j}")
                           for j in range(FG)]
                    for kb in range(KD2):
                        if e == 0 and fg == 0:
                            # interleave x chunk loads with the first
                            # f-group's weight loads so matmul k can start
                            # as soon as its own xt/w01 slices land
                            for c in range(kb * NXC // KD2,
                                           (kb + 1) * NXC // KD2):
                                ring().dma_start(
                                    xt[:, c * XC:(c + 1) * XC, :],
                                    xt_ds[e][c])
                        if (e, kb) in pre01 and fg == 0:
                            w01b = pre01.pop((e, kb))
                        else:
                            w01b = w01_pool.tile([P, KP, 2, FGP], bf16,
                                                 tag="w01b")
                            if e == 0 and fg == 0 and kb == 0:
                                # ramp the very first weight DMA (1/4, 1/4,
                                # 1/2): the first matmul waits on ~256 KB
                                # instead of the full chunk
                                q = KP // 4
                                for (a, b) in ((0, q), (q, 2 * q),
                                               (2 * q, KP)):
                                    ring().dma_start(w01b[:, a:b],
                                                     w01_d[e, fg, kb][:, a:b])
                            else:
                                ring().dma_start(w01b[:], w01_d[e, fg, kb])
                        for kk in range(KP):
                            k = kb * KP + kk
                            for j in range(FG):
                                nc.tensor.matmul(
                                    psG[j][:, :TG],
                                    w01b[:, kk, 0, j * P:(j + 1) * P],
                                    xt[:, k, :],
                                    start=(k == 0), stop=(k == KD - 1))
                                nc.tensor.matmul(
                                    psU[j][:, :TG],
                                    w01b[:, kk, 1, j * P:(j + 1) * P],
                                    xt[:, k, :],
                                    start=(k == 0), stop=(k == KD - 1))
                    for j in range(FG):
                        f = fg * FG + j
                        sig = act_pool.tile([P, 512], f32, tag="sig")
                        ht_f = ht[:, f, :]
                        if use_silu:
                            nc.scalar.activation(
                                sig[:, :TG], psG[j][:, :TG], Silu,
                                scale=s0_sb)
                            nc.vector.tensor_mul(
                                ht_f, sig[:, :TG], psU[j][:, :TG])
                        else:
                            nc.scalar.activation(
                                sig[:, :TG], psG[j][:, :TG],
                                mybir.ActivationFunctionType.Sigmoid,
                                scale=s0_sb)
                            nc.vector.tensor_mul(
                                ht_f, sig[:, :TG], psU[j][:, :TG])
                            nc.vector.tensor_mul(
                                ht_f, ht_f, psG[j][:, :TG])

                # --- post-phase-1 prefetch block (ring FIFO = these gen only
                # after all of this expert's w01 chunk D2Ds, i.e. paced to
                # late phase 1 — no bandwidth stolen from the phase-1 feed).
                # Order: this expert's remaining w2 chunks, then the next
                # expert's xt and fg0 w01 chunks (needed at the next phase-1
                # start, before this expert's y-outs unblock the rings).
                while len(w2bs) < NDS:
                    issue_w2()
                if e + 1 < EPC:
                    nxt = get_xt(e + 1)
                    for c in range(NXC):
                        ring().dma_start(nxt[:, c * XC:(c + 1) * XC, :],
                                         xt_ds[e + 1][c])
                    for kb in range(KD2):
                        w01b = w01_pool.tile([P, KP, 2, FGP], bf16,
                                             tag="w01b")
                        ring().dma_start(w01b[:], w01_d[e + 1, 0, kb])
                        pre01[(e + 1, kb)] = w01b

                # --- phase 2: Y^T = W2 @ HT, scaled per token (free axis) ---
                # w2 128x128 blocks are stationary and ht streams as the
                # moving operand, so the matmul columns scale with the padded
                # token count TG (<=512) instead of the fixed D-slice width.
                for dsi in range(NDS):
                    w2b = w2bs[dsi]
                    for j in range(DSW // P):
                        i = dsi * (DSW // P) + j
                        last = (e == EPC - 1 and dsi == NDS - 1
                                and j == DSW // P - 1)
                        # the very last tile computes in two column halves so
                        # the final scale+store overlaps the final matmuls
                        splits = ([(0, TG // 2 // 8 * 8),
                                   (TG // 2 // 8 * 8, TG)] if last
                                  else [(0, TG)])
                        psY = py_pool.tile([P, 512], f32, tag="py",
                                           name=f"psY_{e}_{i}")
                        ysb = out_pool.tile([P, 512], bf16, tag="ysb")
                        for (ta, tb) in splits:
                            for k in range(KF):
                                nc.tensor.matmul(
                                    psY[:, ta:tb],
                                    w2b[:, k, j * P:(j + 1) * P],
                                    ht[:, k, ta:tb],
                                    start=(k == 0), stop=(k == KF - 1))
                            nc.vector.tensor_mul(
                                ysb[:, ta:tb], psY[:, ta:tb],
                                coefp[:, ta:tb])
                            ring().dma_start(y_ds[e][i][:, ta:tb],
                                             ysb[:, ta:tb])

    nc.compile()
    return nc


def _prep_host(inputs):
    """Host-side dispatch: routing weights, expert->core/slot assignment,
    per-expert token gather, device layouts."""
    import ml_dtypes
    bf16 = ml_dtypes.bfloat16

    x = np.ascontiguousarray(np.asarray(inputs["x"], dtype=np.float32))
    w0 = np.asarray(inputs["w0"], dtype=np.float32)
    w1 = np.asarray(inputs["w1"], dtype=np.float32)
    w2 = np.asarray(inputs["w2"], dtype=np.float32)
    s0 = np.asarray(inputs["s0"], dtype=np.float32)
    s1 = np.asarray(inputs["s1"], dtype=np.float32)
    s2 = np.asarray(inputs["s2"], dtype=np.float32)
    se = np.asarray(inputs["selected_experts"]).astype(np.int64)
    rw = np.asarray(inputs["routing_weights"], dtype=np.float32)

    Tn, Dn = x.shape
    En, DFFn, _ = w0.shape
    KD = Dn // P
    KF = DFFn // P
    KD2 = KD // KP
    NXC = KD // XC
    DSW = min(512, Dn)
    NDS = Dn // DSW
    FG = 2
    FGP = FG * P
    NFG = KF // FG

    # combine weight per (expert, token): sum of routing weights over top-k
    cw = np.zeros((En, Tn), np.float32)
    cols = np.arange(Tn)
    for k in range(se.shape[1]):
        np.add.at(cw, (se[:, k], cols), rw[:, k])

    idx = [np.flatnonzero(cw[e] != 0.0) for e in range(En)]
    counts = np.array([len(i) for i in idx])

    # Sort experts by token count; slot s of core c gets the (s*NCORES+c)-th
    # largest. Per-slot padded width = ceil8(max count in that slot), so the
    # matmul columns track the actual routing load instead of worst-case.
    order = np.argsort(-counts, kind="stable")
    assign = [[int(order[s * NCORES + c]) for s in range(EPC)]
              for c in range(NCORES)]
    TGs = []
    for s in range(EPC):
        mx = max(counts[order[s * NCORES + c]] for c in range(NCORES))
        TGs.append(max(256, int(-(-mx // 8) * 8)))
    NTmax = max(len(_mwidths(TG)) for TG in TGs)

    xT = np.ascontiguousarray(x.T).astype(bf16)  # [D, T]
    in_maps = []
    for c in range(NCORES):
        m = {}
        scv = np.zeros((EPC, P, 1 + NTmax), np.float32)
        w01 = np.empty((EPC, NFG, KD, P, 2, FGP), bf16)
        w2t = np.empty((EPC, NDS, P, KF, DSW), bf16)
        for j in range(EPC):
            e = assign[c][j]
            ids = idx[e]
            TG = TGs[j]
            xt = np.zeros((Dn, TG), bf16)
            xt[:, :len(ids)] = xT[:, ids]
            # xt chunks: [NXC, P, XC, TG] with contiguous partition lines
            m[f"xt{j}"] = np.ascontiguousarray(
                xt.reshape(NXC, XC, P, TG).transpose(0, 2, 1, 3))
            coef = np.zeros(TG, np.float32)
            coef[:len(ids)] = s1[e] * s2[e] * cw[e, ids]
            m[f"coefp{j}"] = np.ascontiguousarray(
                np.broadcast_to(coef, (P, TG)))
            scv[j, :, 0] = s0[e]
            # [D, DFF] -> [NFG, KD, P, FGP] blocks, w0/w1 interleaved
            a = w0[e].T.reshape(KD, P, NFG, FGP).transpose(2, 0, 1, 3)
            b = w1[e].T.reshape(KD, P, NFG, FGP).transpose(2, 0, 1, 3)
            w01[j] = np.stack([a, b], axis=3)
            # w2^T [DFF, D] -> [NDS, P, KF, DSW] (contiguous 8KB lines)
            w2t[j] = w2[e].T.reshape(KF, P, NDS, DSW).transpose(2, 1, 0, 3)
        # w01 k-chunks: [EPC, NFG, KD2, P, KP, 2, FGP] (contiguous lines)
        m["w01"] = np.ascontiguousarray(
            w01.reshape(EPC, NFG, KD2, KP, P, 2, FGP)
               .transpose(0, 1, 2, 4, 3, 5, 6))
        m["w2t"] = w2t
        m["scv"] = scv
        in_maps.append(m)
    return in_maps, idx, assign, tuple(TGs), (Tn, Dn, DFFn)


def _combine(results, idx, assign, shapes):
    """Unshard: scatter-add per-expert outputs into the dense [T, D] output."""
    Tn, Dn, _ = shapes
    out = np.zeros((Tn, Dn), np.float32)
    for c in range(NCORES):
        for j in range(EPC):
            e = assign[c][j]
            ids = idx[e]
            if len(ids):
                # y layout [KD, P, TG] = y^T tiles (D on partitions)
                y = results[c][f"y{j}"]
                yt = y.reshape(Dn, y.shape[-1])[:, :len(ids)]
                out[ids] += yt.T.astype(np.float32)
    return out


def _ensure_axon_ntff_hook():
    """Provide antenv.axon_hooks if the image's antenv stub lacks it.

    concourse.bass_utils imports it unconditionally when BASS_TRACE/trace is
    set under axon; without this the run crashes. When libaxon_pjrt.so exposes
    the NRT-profile symbols we also install the real hook so NTFF profiling
    (HW exec times) works; otherwise tracing degrades to a warning.
    """
    import sys
    import types
    try:
        import antenv.axon_hooks  # noqa: F401
        return
    except ImportError:
        pass
    try:
        import antenv

        mod = types.ModuleType("antenv.axon_hooks")
        _state = {"hook": None}
        mod.set_axon_ntff_profile_hook = lambda h: _state.__setitem__("hook", h)
        mod.get_axon_ntff_profile_hook = lambda: _state["hook"]
        sys.modules["antenv.axon_hooks"] = mod
        antenv.axon_hooks = mod
        try:
            from trn_agent_boot.trn_boot import _ntff_profile_via_ctypes

            so = "/opt/axon/libaxon_pjrt.so"
            if os.path.exists(so):
                mod.set_axon_ntff_profile_hook(_ntff_profile_via_ctypes(so))
        except Exception:
            pass
    except Exception:
        pass


def kernel(**inputs) -> np.ndarray:
    global LAST_RESULT
    _ensure_axon_ntff_hook()
    from concourse.bass_utils import run_bass_kernel_spmd

    in_maps, idx, assign, TGs, shapes = _prep_host(inputs)

    key = TGs + shapes
    nc = _PROGRAM_CACHE.get(key)
    if nc is None:
        nc = _build_program(list(TGs), d=shapes[1], dff=shapes[2])
        _PROGRAM_CACHE[key] = nc

    res = run_bass_kernel_spmd(nc, in_maps, core_ids=list(range(NCORES)))
    LAST_RESULT = res
    return _combine(res.results, idx, assign, shapes)


# revision 38
# speedup vs baseline: 1.0279x; 1.0279x over previous
"""DeepSeek-style MoE block (SwiGLU experts, top-k routing) on 8 Trainium2 cores.

Expert-parallel sharding: each of the 8 cores owns E/8 = 2 experts and receives
only the tokens routed to those experts (host-side dispatch). The device kernel
computes, per expert e with gathered/padded tokens XT [D, TG] (transposed):

    GT = W0e @ X^T          (PSUM, bf16 matmuls, DFF on partitions)
    UT = W1e @ X^T
    HT = silu(s0*GT) * UT   (SBUF, [DFF, TG])
    Y  = (HT)^T @ W2e^T     (tokens on partitions)
    Yout = coef[token] * Y  where coef = s1*s2*cw  (cw = summed routing weights)

The host then scatter-adds each expert's Yout rows into the dense [T, D] output
(the unshard/combine step for expert-parallel sharding).

Perf notes (from NTFF traces):
 - all matmul operands (xt, w01, w2, ht) are bf16: 1 col/cycle warm PE rate
   with half the fp32 DMA traffic, and bf16 enables the compiler's fast
   weight load (FWL) so LDWEIGHTS hides under the matmul stream (fp32
   LDWEIGHTS at ~198 ns/load nearly serialized with the MMs). PSUM
   accumulation and the activation path stay fp32.
 - Experts are sorted by routed-token count and paired big+small onto
   cores: slot 0 pads to the max big count, slot 1 to the max small count
   (ceil-8), trimming ~10% of the padded matmul columns vs uniform 384.
 - TRN2 has two physical HWDGE rings (SP + Activation). All large DMAs
   alternate between nc.sync and nc.scalar so the two rings run in parallel.
 - DMA pacing is everything: the 16 SDMA engines round-robin over in-flight
   transfers at packet granularity, so completions are fair-shared, not
   issue-ordered. w01 streams as 512 KB 4-k-tile chunks (big enough that the
   first chunk isn't starved by its siblings, small enough not to delay the
   first matmul), and each expert's w2 + the next expert's xt/fg0-w01 are
   issued behind the full w01 stream on the ring FIFO so they cannot steal
   HBM bandwidth from the critical phase-1 feed.
 - Split PSUM pools (6 banks phase 1 + 2 banks phase 2) so the phases never
   chain on each other's bank releases at expert/phase boundaries.
"""

import os
import numpy as np

T, D, DFF, E, TOPK = 1024, 2048, 1024, 16, 6
NCORES, P = 8, 128
EPC = E // NCORES  # experts per core

# Set by kernel() after each run: BassKernelResults (exec_time_ns when traced).
LAST_RESULT = None

_PROGRAM_CACHE = {}

XC = 4   # xt k-tiles per DMA chunk
KP = 4   # w01 k-tiles per DMA chunk


def _mwidths(TG):
    """Token-tile widths for phase 2 (tokens become PSUM partitions)."""
    out = []
    left = TG
    while left > 0:
        out.append(min(P, left))
        left -= P
    return out


def _build_program(TGs, d=D, dff=DFF, use_silu=True):
    """Build + compile the SPMD single-core Bass program.

    TGs: per-slot padded token counts (multiples of 8, <= 512).
    use_silu=False decomposes silu into sigmoid+mul (CoreSim lacks Silu).
    """
    import concourse.bacc as bacc
    import concourse.mybir as mybir
    import concourse.tile as tile

    f32 = mybir.dt.float32
    bf16 = mybir.dt.bfloat16
    Silu = mybir.ActivationFunctionType.Silu

    KD = d // P        # k-tiles over D (contraction of W0/W1 matmuls)
    KF = dff // P      # k-tiles over DFF (contraction of W2 matmul)
    DSW = min(512, d)  # output D slice width
    NDS = d // DSW     # output D slices
    FG = 2             # DFF f-tiles per PSUM group (2G + 2U = 4 banks/group)
    FGP = FG * P
    NFG = KF // FG
    NXC = KD // XC
    KD2 = KD // KP
    NTmax = max(len(_mwidths(TG)) for TG in TGs)

    assert all(TG <= 512 for TG in TGs)

    nc = bacc.Bacc("TRN2", target_bir_lowering=False, debug=False)

    xt_ds = [nc.dram_tensor(f"xt{e}", [NXC, P, XC, TGs[e]], bf16,
                            kind="ExternalInput").ap() for e in range(EPC)]
    w01_d = nc.dram_tensor("w01", [EPC, NFG, KD2, P, KP, 2, FGP], bf16,
                           kind="ExternalInput").ap()
    w2t_d = nc.dram_tensor("w2t", [EPC, NDS, P, KF, DSW], bf16,
                           kind="ExternalInput").ap()
    sc_d = nc.dram_tensor("scv", [EPC, P, 1 + NTmax], f32,
                          kind="ExternalInput").ap()
    # coef replicated across partitions: phase 2 scales PSUM columns (tokens)
    # during the PSUM->SBUF cast, so the per-token combine weight needs to be
    # available on every output partition.
    cp_ds = [nc.dram_tensor(f"coefp{e}", [P, TGs[e]], f32,
                            kind="ExternalInput").ap() for e in range(EPC)]
    # y layout [KD, P, TG]: phase-2 output has D on partitions, tokens free
    y_ds = [nc.dram_tensor(f"y{e}", [KD, P, TGs[e]], bf16,
                           kind="ExternalOutput").ap() for e in range(EPC)]

    with tile.TileContext(nc) as tc:
        # Alternate large DMAs across the two physical HWDGE rings.
        rings = [nc.sync, nc.scalar]
        ring_state = [0]

        def ring():
            ring_state[0] ^= 1
            return rings[ring_state[0]]

        with (
            tc.tile_pool(name="xt", bufs=1) as xt_pool,
            tc.tile_pool(name="w01", bufs=5) as w01_pool,
            tc.tile_pool(name="w2", bufs=4) as w2_pool,
            tc.tile_pool(name="ht", bufs=1) as ht_pool,
            tc.tile_pool(name="act", bufs=6) as act_pool,
            tc.tile_pool(name="out", bufs=8) as out_pool,
            tc.tile_pool(name="sc", bufs=2) as sc_pool,
            tc.tile_pool(name="pgu", bufs=6, space="PSUM") as pgu_pool,
            tc.tile_pool(name="py", bufs=2, space="PSUM") as py_pool,
        ):
            # Tiles created by the previous expert's prefetch block (the ring
            # FIFO paces their descriptor-gen behind the in-order w01 stream,
            # so the prefetch costs no early bandwidth).
            xts = {}        # e -> xt tile (per-slot tag: bufs=1 suffices)
            pre01 = {}      # (e, kb) -> fg0 w01 chunk tile

            def get_xt(e):
                if e not in xts:
                    xts[e] = xt_pool.tile([P, KD, TGs[e]], bf16, tag=f"xt{e}",
                                          name=f"xt_{e}")
                return xts[e]

            for e in range(EPC):
                TG = TGs[e]

                # --- inputs for this expert ---
                xt = get_xt(e)
                scv = sc_pool.tile([P, 1 + NTmax], f32, tag="scv")
                nc.gpsimd.dma_start(scv[:], sc_d[e])
                s0_sb = scv[:, 0:1]
                coefp = sc_pool.tile([P, TG], f32, tag=f"cp{e}",
                                     name=f"coefp_{e}")
                nc.gpsimd.dma_start(coefp[:], cp_ds[e])

                # --- phase 1: HT = silu(s0 * W0 xT) * (W1 xT), [DFF, TG] ---
                # w2 chunks are issued at f-group boundaries (fg>=1): the ring
                # FIFO paces them behind that f-group's w01 stream, spreading
                # the ~4 MB w2 prefetch across phase 1 instead of cramming it
                # into the (now short) phase-2 window — while keeping it out
                # of the cold-start fg0 window.
                w2bs = []

                def issue_w2():
                    dsi = len(w2bs)
                    w2b = w2_pool.tile([P, KF, DSW], bf16, tag="w2b",
                                       name=f"w2b_{e}_{dsi}")
                    ring().dma_start(w2b[:], w2t_d[e, dsi])
                    w2bs.append(w2b)

                ht = ht_pool.tile([P, KF, TG], bf16, tag=f"ht{e}")
                for fg in range(NFG):
                    if fg >= 1 and len(w2bs) < NDS:
                        issue_w2()
                    psG = [pgu_pool.tile([P, 512], f32, tag="pgu",
                                         name=f"psG_{e}_{fg}_{j}")
                           for j in range(FG)]
                    psU = [pgu_pool.tile([P, 512], f32, tag="pgu",
                                         name=f"psU_{e}_{fg}_{j}")
                           for j in range(FG)]
                    for kb in range(KD2):
                        if e == 0 and fg == 0:
                            # interleave x chunk loads with the first
                            # f-group's weight loads so matmul k can start
                            # as soon as its own xt/w01 slices land; the
                            # first chunk is halved so matmul k=0 waits on
                            # ~180 KB of x instead of ~360 KB
                            for c in range(kb * NXC // KD2,
                                           (kb + 1) * NXC // KD2):
                                if c == 0:
                                    h = XC // 2
                                    ring().dma_start(
                                        xt[:, :h, :], xt_ds[e][0][:, :h])
                                    ring().dma_start(
                                        xt[:, h:XC, :], xt_ds[e][0][:, h:])
                                else:
                                    ring().dma_start(
                                        xt[:, c * XC:(c + 1) * XC, :],
                                        xt_ds[e][c])
                        if (e, kb) in pre01 and fg == 0:
                            w01b = pre01.pop((e, kb))
                        else:
                            w01b = w01_pool.tile([P, KP, 2, FGP], bf16,
                                                 tag="w01b")
                            if e == 0 and fg == 0 and kb == 0:
                                # halve the very first weight DMA: the first
                                # matmul waits on ~256 KB instead of 512 KB
                                h = KP // 2
                                ring().dma_start(w01b[:, :h],
                                                 w01_d[e, fg, kb][:, :h])
                                ring().dma_start(w01b[:, h:],
                                                 w01_d[e, fg, kb][:, h:])
                            else:
                                ring().dma_start(w01b[:], w01_d[e, fg, kb])
                        for kk in range(KP):
                            k = kb * KP + kk
                            for j in range(FG):
                                nc.tensor.matmul(
                                    psG[j][:, :TG],
                                    w01b[:, kk, 0, j * P:(j + 1) * P],
                                    xt[:, k, :],
                                    start=(k == 0), stop=(k == KD - 1))
                                nc.tensor.matmul(
                                    psU[j][:, :TG],
                                    w01b[:, kk, 1, j * P:(j + 1) * P],
                                    xt[:, k, :],
                                    start=(k == 0), stop=(k == KD - 1))
                    for j in range(FG):
                        f = fg * FG + j
                        sig = act_pool.tile([P, 512], f32, tag="sig")
                        ht_f = ht[:, f, :]
                        if use_silu:
                            nc.scalar.activation(
                                sig[:, :TG], psG[j][:, :TG], Silu,
                                scale=s0_sb)
                            nc.vector.tensor_mul(
                                ht_f, sig[:, :TG], psU[j][:, :TG])
                        else:
                            nc.scalar.activation(
                                sig[:, :TG], psG[j][:, :TG],
                                mybir.ActivationFunctionType.Sigmoid,
                                scale=s0_sb)
                            nc.vector.tensor_mul(
                                ht_f, sig[:, :TG], psU[j][:, :TG])
                            nc.vector.tensor_mul(
                                ht_f, ht_f, psG[j][:, :TG])

                # --- post-phase-1 prefetch block (ring FIFO = these gen only
                # after all of this expert's w01 chunk D2Ds, i.e. paced to
                # late phase 1 — no bandwidth stolen from the phase-1 feed).
                # Order: this expert's remaining w2 chunks, then the next
                # expert's xt and fg0 w01 chunks (needed at the next phase-1
                # start, before this expert's y-outs unblock the rings).
                while len(w2bs) < NDS:
                    issue_w2()
                if e + 1 < EPC:
                    nxt = get_xt(e + 1)
                    for c in range(NXC):
                        ring().dma_start(nxt[:, c * XC:(c + 1) * XC, :],
                                         xt_ds[e + 1][c])
                    for kb in range(KD2):
                        w01b = w01_pool.tile([P, KP, 2, FGP], bf16,
                                             tag="w01b")
                        ring().dma_start(w01b[:], w01_d[e + 1, 0, kb])
                        pre01[(e + 1, kb)] = w01b

                # --- phase 2: Y^T = W2 @ HT, scaled per token (free axis) ---
                # w2 128x128 blocks are stationary and ht streams as the
                # moving operand, so the matmul columns scale with the padded
                # token count TG (<=512) instead of the fixed D-slice width.
                for dsi in range(NDS):
                    w2b = w2bs[dsi]
                    for j in range(DSW // P):
                        i = dsi * (DSW // P) + j
                        last = (e == EPC - 1 and dsi == NDS - 1
                                and j == DSW // P - 1)
                        # the very last tile computes in two column halves so
                        # the final scale+store overlaps the final matmuls
                        splits = ([(0, TG // 2 // 8 * 8),
                                   (TG // 2 // 8 * 8, TG)] if last
                                  else [(0, TG)])
                        psY = py_pool.tile([P, 512], f32, tag="py",
                                           name=f"psY_{e}_{i}")
                        ysb = out_pool.tile([P, 512], bf16, tag="ysb")
                        for (ta, tb) in splits:
                            for k in range(KF):
                                nc.tensor.matmul(
                                    psY[:, ta:tb],
                                    w2b[:, k, j * P:(j + 1) * P],
                                    ht[:, k, ta:tb],
                                    start=(k == 0), stop=(k == KF - 1))
                            nc.vector.tensor_mul(
                                ysb[:, ta:tb], psY[:, ta:tb],
                                coefp[:, ta:tb])
                            ring().dma_start(y_ds[e][i][:, ta:tb],
                                             ysb[:, ta:tb])

    nc.compile()
    return nc


def _prep_host(inputs):
    """Host-side dispatch: routing weights, expert->core/slot assignment,
    per-expert token gather, device layouts."""
    import ml_dtypes
    bf16 = ml_dtypes.bfloat16

    x = np.ascontiguousarray(np.asarray(inputs["x"], dtype=np.float32))
    w0 = np.asarray(inputs["w0"], dtype=np.float32)
    w1 = np.asarray(inputs["w1"], dtype=np.float32)
    w2 = np.asarray(inputs["w2"], dtype=np.float32)
    s0 = np.asarray(inputs["s0"], dtype=np.float32)
    s1 = np.asarray(inputs["s1"], dtype=np.float32)
    s2 = np.asarray(inputs["s2"], dtype=np.float32)
    se = np.asarray(inputs["selected_experts"]).astype(np.int64)
    rw = np.asarray(inputs["routing_weights"], dtype=np.float32)

    Tn, Dn = x.shape
    En, DFFn, _ = w0.shape
    KD = Dn // P
    KF = DFFn // P
    KD2 = KD // KP
    NXC = KD // XC
    DSW = min(512, Dn)
    NDS = Dn // DSW
    FG = 2
    FGP = FG * P
    NFG = KF // FG

    # combine weight per (expert, token): sum of routing weights over top-k
    cw = np.zeros((En, Tn), np.float32)
    cols = np.arange(Tn)
    for k in range(se.shape[1]):
        np.add.at(cw, (se[:, k], cols), rw[:, k])

    idx = [np.flatnonzero(cw[e] != 0.0) for e in range(En)]
    counts = np.array([len(i) for i in idx])

    # Sort experts by token count; slot s of core c gets the (s*NCORES+c)-th
    # largest. Per-slot padded width = ceil8(max count in that slot), so the
    # matmul columns track the actual routing load instead of worst-case.
    order = np.argsort(-counts, kind="stable")
    assign = [[int(order[s * NCORES + c]) for s in range(EPC)]
              for c in range(NCORES)]
    TGs = []
    for s in range(EPC):
        mx = max(counts[order[s * NCORES + c]] for c in range(NCORES))
        TGs.append(max(256, int(-(-mx // 8) * 8)))
    NTmax = max(len(_mwidths(TG)) for TG in TGs)

    xT = np.ascontiguousarray(x.T).astype(bf16)  # [D, T]
    in_maps = []
    for c in range(NCORES):
        m = {}
        scv = np.zeros((EPC, P, 1 + NTmax), np.float32)
        w01 = np.empty((EPC, NFG, KD, P, 2, FGP), bf16)
        w2t = np.empty((EPC, NDS, P, KF, DSW), bf16)
        for j in range(EPC):
            e = assign[c][j]
            ids = idx[e]
            TG = TGs[j]
            xt = np.zeros((Dn, TG), bf16)
            xt[:, :len(ids)] = xT[:, ids]
            # xt chunks: [NXC, P, XC, TG] with contiguous partition lines
            m[f"xt{j}"] = np.ascontiguousarray(
                xt.reshape(NXC, XC, P, TG).transpose(0, 2, 1, 3))
            coef = np.zeros(TG, np.float32)
            coef[:len(ids)] = s1[e] * s2[e] * cw[e, ids]
            m[f"coefp{j}"] = np.ascontiguousarray(
                np.broadcast_to(coef, (P, TG)))
            scv[j, :, 0] = s0[e]
            # [D, DFF] -> [NFG, KD, P, FGP] blocks, w0/w1 interleaved
            a = w0[e].T.reshape(KD, P, NFG, FGP).transpose(2, 0, 1, 3)
            b = w1[e].T.reshape(KD, P, NFG, FGP).transpose(2, 0, 1, 3)
            w01[j] = np.stack([a, b], axis=3)
            # w2^T [DFF, D] -> [NDS, P, KF, DSW] (contiguous 8KB lines)
            w2t[j] = w2[e].T.reshape(KF, P, NDS, DSW).transpose(2, 1, 0, 3)
        # w01 k-chunks: [EPC, NFG, KD2, P, KP, 2, FGP] (contiguous lines)
        m["w01"] = np.ascontiguousarray(
            w01.reshape(EPC, NFG, KD2, KP, P, 2, FGP)
               .transpose(0, 1, 2, 4, 3, 5, 6))
        m["w2t"] = w2t
        m["scv"] = scv
        in_maps.append(m)
    return in_maps, idx, assign, tuple(TGs), (Tn, Dn, DFFn)


def _combine(results, idx, assign, shapes):
    """Unshard: scatter-add per-expert outputs into the dense [T, D] output."""
    Tn, Dn, _ = shapes
    out = np.zeros((Tn, Dn), np.float32)
    for c in range(NCORES):
        for j in range(EPC):
            e = assign[c][j]
            ids = idx[e]
            if len(ids):
                # y layout [KD, P, TG] = y^T tiles (D on partitions)
                y = results[c][f"y{j}"]
                yt = y.reshape(Dn, y.shape[-1])[:, :len(ids)]
                out[ids] += yt.T.astype(np.float32)
    return out


def _ensure_axon_ntff_hook():
    """Provide antenv.axon_hooks if the image's antenv stub lacks it.

    concourse.bass_utils imports it unconditionally when BASS_TRACE/trace is
    set under axon; without this the run crashes. When libaxon_pjrt.so exposes
    the NRT-profile symbols we also install the real hook so NTFF profiling
    (HW exec times) works; otherwise tracing degrades to a warning.
    """
    import sys
    import types
    try:
        import antenv.axon_hooks  # noqa: F401
        return
    except ImportError:
        pass
    try:
        import antenv

        mod = types.ModuleType("antenv.axon_hooks")
        _state = {"hook": None}
        mod.set_axon_ntff_profile_hook = lambda h: _state.__setitem__("hook", h)
        mod.get_axon_ntff_profile_hook = lambda: _state["hook"]
        sys.modules["antenv.axon_hooks"] = mod
        antenv.axon_hooks = mod
        try:
            from trn_agent_boot.trn_boot import _ntff_profile_via_ctypes

            so = "/opt/axon/libaxon_pjrt.so"
            if os.path.exists(so):
                mod.set_axon_ntff_profile_hook(_ntff_profile_via_ctypes(so))
        except Exception:
            pass
    except Exception:
        pass


def kernel(**inputs) -> np.ndarray:
    global LAST_RESULT
    _ensure_axon_ntff_hook()
    from concourse.bass_utils import run_bass_kernel_spmd

    in_maps, idx, assign, TGs, shapes = _prep_host(inputs)

    key = TGs + shapes
    nc = _PROGRAM_CACHE.get(key)
    if nc is None:
        nc = _build_program(list(TGs), d=shapes[1], dff=shapes[2])
        _PROGRAM_CACHE[key] = nc

    res = run_bass_kernel_spmd(nc, in_maps, core_ids=list(range(NCORES)))
    LAST_RESULT = res
    return _combine(res.results, idx, assign, shapes)


# revision 39
# speedup vs baseline: 1.0359x; 1.0077x over previous
"""DeepSeek-style MoE block (SwiGLU experts, top-k routing) on 8 Trainium2 cores.

Expert-parallel sharding: each of the 8 cores owns E/8 = 2 experts and receives
only the tokens routed to those experts (host-side dispatch). The device kernel
computes, per expert e with gathered/padded tokens XT [D, TG] (transposed):

    GT = W0e @ X^T             (PSUM, bf16 matmuls, DFF on partitions)
    UT = W1e @ X^T
    HT = silu(s0*GT) * UT      (SBUF bf16, [DFF, TG])
    Y^T = (W2e @ HT) * coef    (D on partitions, tokens on the free axis,
                                coef = s1*s2*cw broadcast to all partitions,
                                applied during the PSUM->SBUF cast)

The host then scatter-adds each expert's Y columns into the dense [T, D]
output (the unshard/combine step for expert-parallel sharding).

Perf notes (from NTFF traces; baseline fp32 kernel was 210.7 us, this one
measures ~140-146 us on hardware):
 - all matmul operands (xt, w01, w2, ht) are bf16: 1 col/cycle warm PE rate
   with half the fp32 DMA traffic, and bf16 enables the compiler's fast
   weight load (FWL) so LDWEIGHTS hides under the matmul stream (fp32
   LDWEIGHTS at ~198 ns/load nearly serialized with the MMs). PSUM
   accumulation and the activation path stay fp32.
 - Experts are sorted by routed-token count and paired big+small onto
   cores: slot 0 pads to the max big count, slot 1 to the max small count
   (ceil-8). Phase 2 streams ht (TG token columns) against stationary w2
   128x128 blocks, so BOTH phases' matmul columns scale with the actual
   routing load: ~10% fewer PE cycles than uniform 384-token padding.
 - TRN2 has two physical HWDGE rings (SP + Activation). All large DMAs
   alternate between nc.sync and nc.scalar so the two rings run in parallel.
 - DMA pacing is everything. The 16 SDMA engines round-robin over in-flight
   transfers at packet granularity, so completions are fair-shared, not
   issue-ordered: many small eager loads starve the one the PE needs first.
   w01 streams as 512 KB 4-k-tile chunks (the first one ramped 256+256 KB so
   matmul k=0 starts early); w2 chunks are issued at f-group boundaries
   fg>=1 so they spread across phase 1 without touching the cold-start
   window; the next expert's xt and fg0 w01 ride the ring FIFO behind the
   current expert's full w01 stream. Pool bufs (w01=5, w2=4) are sized so no
   ring-head tile-slot wait ever blocks a later-needed stream.
 - Split PSUM pools (6 banks phase 1 + 2 banks phase 2) so the phases never
   chain on each other's bank releases at expert/phase boundaries.
 - Remaining span: ~7 us fixed NEFF preamble + ~5 us first-data wait, then
   the fg0-fg2 window of expert 0 rides the HBM roofline (engines ~100%
   busy) with the PE HAM-cold underneath, then PE-bound warm matmuls to the
   end; ~5 us tail = last store + fixed teardown.
"""

import os
import numpy as np

T, D, DFF, E, TOPK = 1024, 2048, 1024, 16, 6
NCORES, P = 8, 128
EPC = E // NCORES  # experts per core

# Set by kernel() after each run: BassKernelResults (exec_time_ns when traced).
LAST_RESULT = None

_PROGRAM_CACHE = {}

XC = 4   # xt k-tiles per DMA chunk
KP = 4   # w01 k-tiles per DMA chunk


def _mwidths(TG):
    """Token-tile widths for phase 2 (tokens become PSUM partitions)."""
    out = []
    left = TG
    while left > 0:
        out.append(min(P, left))
        left -= P
    return out


def _build_program(TGs, d=D, dff=DFF, use_silu=True):
    """Build + compile the SPMD single-core Bass program.

    TGs: per-slot padded token counts (multiples of 8, <= 512).
    use_silu=False decomposes silu into sigmoid+mul (CoreSim lacks Silu).
    """
    import concourse.bacc as bacc
    import concourse.mybir as mybir
    import concourse.tile as tile

    f32 = mybir.dt.float32
    bf16 = mybir.dt.bfloat16
    Silu = mybir.ActivationFunctionType.Silu

    KD = d // P        # k-tiles over D (contraction of W0/W1 matmuls)
    KF = dff // P      # k-tiles over DFF (contraction of W2 matmul)
    DSW = min(512, d)  # output D slice width
    NDS = d // DSW     # output D slices
    FG = 2             # DFF f-tiles per PSUM group (2G + 2U = 4 banks/group)
    FGP = FG * P
    NFG = KF // FG
    NXC = KD // XC
    KD2 = KD // KP
    NTmax = max(len(_mwidths(TG)) for TG in TGs)

    assert all(TG <= 512 for TG in TGs)

    nc = bacc.Bacc("TRN2", target_bir_lowering=False, debug=False)

    xt_ds = [nc.dram_tensor(f"xt{e}", [NXC, P, XC, TGs[e]], bf16,
                            kind="ExternalInput").ap() for e in range(EPC)]
    w01_d = nc.dram_tensor("w01", [EPC, NFG, KD2, P, KP, 2, FGP], bf16,
                           kind="ExternalInput").ap()
    w2t_d = nc.dram_tensor("w2t", [EPC, NDS, P, KF, DSW], bf16,
                           kind="ExternalInput").ap()
    sc_d = nc.dram_tensor("scv", [EPC, P, 1 + NTmax], f32,
                          kind="ExternalInput").ap()
    # coef replicated across partitions: phase 2 scales PSUM columns (tokens)
    # during the PSUM->SBUF cast, so the per-token combine weight needs to be
    # available on every output partition.
    cp_ds = [nc.dram_tensor(f"coefp{e}", [P, TGs[e]], f32,
                            kind="ExternalInput").ap() for e in range(EPC)]
    # y layout [KD, P, TG]: phase-2 output has D on partitions, tokens free
    y_ds = [nc.dram_tensor(f"y{e}", [KD, P, TGs[e]], bf16,
                           kind="ExternalOutput").ap() for e in range(EPC)]

    with tile.TileContext(nc) as tc:
        # Alternate large DMAs across the two physical HWDGE rings.
        rings = [nc.sync, nc.scalar]
        ring_state = [0]

        def ring():
            ring_state[0] ^= 1
            return rings[ring_state[0]]

        with (
            tc.tile_pool(name="xt", bufs=1) as xt_pool,
            tc.tile_pool(name="w01", bufs=5) as w01_pool,
            tc.tile_pool(name="w2", bufs=4) as w2_pool,
            tc.tile_pool(name="ht", bufs=1) as ht_pool,
            tc.tile_pool(name="act", bufs=6) as act_pool,
            tc.tile_pool(name="out", bufs=8) as out_pool,
            tc.tile_pool(name="sc", bufs=2) as sc_pool,
            tc.tile_pool(name="pgu", bufs=6, space="PSUM") as pgu_pool,
            tc.tile_pool(name="py", bufs=2, space="PSUM") as py_pool,
        ):
            # Tiles created by the previous expert's prefetch block (the ring
            # FIFO paces their descriptor-gen behind the in-order w01 stream,
            # so the prefetch costs no early bandwidth).
            xts = {}        # e -> xt tile (per-slot tag: bufs=1 suffices)
            pre01 = {}      # (e, kb) -> fg0 w01 chunk tile

            def get_xt(e):
                if e not in xts:
                    xts[e] = xt_pool.tile([P, KD, TGs[e]], bf16, tag=f"xt{e}",
                                          name=f"xt_{e}")
                return xts[e]

            for e in range(EPC):
                TG = TGs[e]

                # --- inputs for this expert ---
                xt = get_xt(e)
                scv = sc_pool.tile([P, 1 + NTmax], f32, tag="scv")
                nc.gpsimd.dma_start(scv[:], sc_d[e])
                s0_sb = scv[:, 0:1]
                coefp = sc_pool.tile([P, TG], f32, tag=f"cp{e}",
                                     name=f"coefp_{e}")
                nc.gpsimd.dma_start(coefp[:], cp_ds[e])

                # --- phase 1: HT = silu(s0 * W0 xT) * (W1 xT), [DFF, TG] ---
                # w2 chunks are issued at f-group boundaries (fg>=1): the ring
                # FIFO paces them behind that f-group's w01 stream, spreading
                # the ~4 MB w2 prefetch across phase 1 instead of cramming it
                # into the (now short) phase-2 window — while keeping it out
                # of the cold-start fg0 window.
                w2bs = []

                def issue_w2():
                    dsi = len(w2bs)
                    w2b = w2_pool.tile([P, KF, DSW], bf16, tag="w2b",
                                       name=f"w2b_{e}_{dsi}")
                    ring().dma_start(w2b[:], w2t_d[e, dsi])
                    w2bs.append(w2b)

                ht = ht_pool.tile([P, KF, TG], bf16, tag=f"ht{e}")
                for fg in range(NFG):
                    if fg >= 1 and len(w2bs) < NDS:
                        issue_w2()
                    psG = [pgu_pool.tile([P, 512], f32, tag="pgu",
                                         name=f"psG_{e}_{fg}_{j}")
                           for j in range(FG)]
                    psU = [pgu_pool.tile([P, 512], f32, tag="pgu",
                                         name=f"psU_{e}_{fg}_{j}")
                           for j in range(FG)]
                    for kb in range(KD2):
                        if e == 0 and fg == 0:
                            # interleave x chunk loads with the first
                            # f-group's weight loads so matmul k can start
                            # as soon as its own xt/w01 slices land; the
                            # first chunk is halved so matmul k=0 waits on
                            # ~180 KB of x instead of ~360 KB
                            for c in range(kb * NXC // KD2,
                                           (kb + 1) * NXC // KD2):
                                if c == 0:
                                    h = XC // 2
                                    ring().dma_start(
                                        xt[:, :h, :], xt_ds[e][0][:, :h])
                                    ring().dma_start(
                                        xt[:, h:XC, :], xt_ds[e][0][:, h:])
                                else:
                                    ring().dma_start(
                                        xt[:, c * XC:(c + 1) * XC, :],
                                        xt_ds[e][c])
                        if (e, kb) in pre01 and fg == 0:
                            w01b = pre01.pop((e, kb))
                        else:
                            w01b = w01_pool.tile([P, KP, 2, FGP], bf16,
                                                 tag="w01b")
                            if e == 0 and fg == 0 and kb == 0:
                                # halve the very first weight DMA: the first
                                # matmul waits on ~256 KB instead of 512 KB
                                h = KP // 2
                                ring().dma_start(w01b[:, :h],
                                                 w01_d[e, fg, kb][:, :h])
                                ring().dma_start(w01b[:, h:],
                                                 w01_d[e, fg, kb][:, h:])
                            else:
                                ring().dma_start(w01b[:], w01_d[e, fg, kb])
                        for kk in range(KP):
                            k = kb * KP + kk
                            for j in range(FG):
                                nc.tensor.matmul(
                                    psG[j][:, :TG],
                                    w01b[:, kk, 0, j * P:(j + 1) * P],
                                    xt[:, k, :],
                                    start=(k == 0), stop=(k == KD - 1))
                                nc.tensor.matmul(
                                    psU[j][:, :TG],
                                    w01b[:, kk, 1, j * P:(j + 1) * P],
                                    xt[:, k, :],
                                    start=(k == 0), stop=(k == KD - 1))
                    for j in range(FG):
                        f = fg * FG + j
                        sig = act_pool.tile([P, 512], f32, tag="sig")
                        ht_f = ht[:, f, :]
                        if use_silu:
                            nc.scalar.activation(
                                sig[:, :TG], psG[j][:, :TG], Silu,
                                scale=s0_sb)
                            nc.vector.tensor_mul(
                                ht_f, sig[:, :TG], psU[j][:, :TG])
                        else:
                            nc.scalar.activation(
                                sig[:, :TG], psG[j][:, :TG],
                                mybir.ActivationFunctionType.Sigmoid,
                                scale=s0_sb)
                            nc.vector.tensor_mul(
                                ht_f, sig[:, :TG], psU[j][:, :TG])
                            nc.vector.tensor_mul(
                                ht_f, ht_f, psG[j][:, :TG])

                # --- post-phase-1 prefetch block (ring FIFO = these gen only
                # after all of this expert's w01 chunk D2Ds, i.e. paced to
                # late phase 1 — no bandwidth stolen from the phase-1 feed).
                # Order: this expert's remaining w2 chunks, then the next
                # expert's xt and fg0 w01 chunks (needed at the next phase-1
                # start, before this expert's y-outs unblock the rings).
                while len(w2bs) < NDS:
                    issue_w2()
                if e + 1 < EPC:
                    nxt = get_xt(e + 1)
                    for c in range(NXC):
                        ring().dma_start(nxt[:, c * XC:(c + 1) * XC, :],
                                         xt_ds[e + 1][c])
                    for kb in range(KD2):
                        w01b = w01_pool.tile([P, KP, 2, FGP], bf16,
                                             tag="w01b")
                        ring().dma_start(w01b[:], w01_d[e + 1, 0, kb])
                        pre01[(e + 1, kb)] = w01b

                # --- phase 2: Y^T = W2 @ HT, scaled per token (free axis) ---
                # w2 128x128 blocks are stationary and ht streams as the
                # moving operand, so the matmul columns scale with the padded
                # token count TG (<=512) instead of the fixed D-slice width.
                for dsi in range(NDS):
                    w2b = w2bs[dsi]
                    for j in range(DSW // P):
                        i = dsi * (DSW // P) + j
                        last = (e == EPC - 1 and dsi == NDS - 1
                                and j == DSW // P - 1)
                        # the very last tile computes in two column halves so
                        # the final scale+store overlaps the final matmuls
                        splits = ([(0, TG // 2 // 8 * 8),
                                   (TG // 2 // 8 * 8, TG)] if last
                                  else [(0, TG)])
                        psY = py_pool.tile([P, 512], f32, tag="py",
                                           name=f"psY_{e}_{i}")
                        ysb = out_pool.tile([P, 512], bf16, tag="ysb")
                        for (ta, tb) in splits:
                            for k in range(KF):
                                nc.tensor.matmul(
                                    psY[:, ta:tb],
                                    w2b[:, k, j * P:(j + 1) * P],
                                    ht[:, k, ta:tb],
                                    start=(k == 0), stop=(k == KF - 1))
                            nc.vector.tensor_mul(
                                ysb[:, ta:tb], psY[:, ta:tb],
                                coefp[:, ta:tb])
                            ring().dma_start(y_ds[e][i][:, ta:tb],
                                             ysb[:, ta:tb])

    nc.compile()
    return nc


def _prep_host(inputs):
    """Host-side dispatch: routing weights, expert->core/slot assignment,
    per-expert token gather, device layouts."""
    import ml_dtypes
    bf16 = ml_dtypes.bfloat16

    x = np.ascontiguousarray(np.asarray(inputs["x"], dtype=np.float32))
    w0 = np.asarray(inputs["w0"], dtype=np.float32)
    w1 = np.asarray(inputs["w1"], dtype=np.float32)
    w2 = np.asarray(inputs["w2"], dtype=np.float32)
    s0 = np.asarray(inputs["s0"], dtype=np.float32)
    s1 = np.asarray(inputs["s1"], dtype=np.float32)
    s2 = np.asarray(inputs["s2"], dtype=np.float32)
    se = np.asarray(inputs["selected_experts"]).astype(np.int64)
    rw = np.asarray(inputs["routing_weights"], dtype=np.float32)

    Tn, Dn = x.shape
    En, DFFn, _ = w0.shape
    KD = Dn // P
    KF = DFFn // P
    KD2 = KD // KP
    NXC = KD // XC
    DSW = min(512, Dn)
    NDS = Dn // DSW
    FG = 2
    FGP = FG * P
    NFG = KF // FG

    # combine weight per (expert, token): sum of routing weights over top-k
    cw = np.zeros((En, Tn), np.float32)
    cols = np.arange(Tn)
    for k in range(se.shape[1]):
        np.add.at(cw, (se[:, k], cols), rw[:, k])

    idx = [np.flatnonzero(cw[e] != 0.0) for e in range(En)]
    counts = np.array([len(i) for i in idx])

    # Sort experts by token count; slot s of core c gets the (s*NCORES+c)-th
    # largest. Per-slot padded width = ceil8(max count in that slot), so the
    # matmul columns track the actual routing load instead of worst-case.
    order = np.argsort(-counts, kind="stable")
    assign = [[int(order[s * NCORES + c]) for s in range(EPC)]
              for c in range(NCORES)]
    TGs = []
    for s in range(EPC):
        mx = max(counts[order[s * NCORES + c]] for c in range(NCORES))
        TGs.append(max(256, int(-(-mx // 8) * 8)))
    NTmax = max(len(_mwidths(TG)) for TG in TGs)

    xT = np.ascontiguousarray(x.T).astype(bf16)  # [D, T]
    in_maps = []
    for c in range(NCORES):
        m = {}
        scv = np.zeros((EPC, P, 1 + NTmax), np.float32)
        w01 = np.empty((EPC, NFG, KD, P, 2, FGP), bf16)
        w2t = np.empty((EPC, NDS, P, KF, DSW), bf16)
        for j in range(EPC):
            e = assign[c][j]
            ids = idx[e]
            TG = TGs[j]
            xt = np.zeros((Dn, TG), bf16)
            xt[:, :len(ids)] = xT[:, ids]
            # xt chunks: [NXC, P, XC, TG] with contiguous partition lines
            m[f"xt{j}"] = np.ascontiguousarray(
                xt.reshape(NXC, XC, P, TG).transpose(0, 2, 1, 3))
            coef = np.zeros(TG, np.float32)
            coef[:len(ids)] = s1[e] * s2[e] * cw[e, ids]
            m[f"coefp{j}"] = np.ascontiguousarray(
                np.broadcast_to(coef, (P, TG)))
            scv[j, :, 0] = s0[e]
            # [D, DFF] -> [NFG, KD, P, FGP] blocks, w0/w1 interleaved
            a = w0[e].T.reshape(KD, P, NFG, FGP).transpose(2, 0, 1, 3)
            b = w1[e].T.reshape(KD, P, NFG, FGP).transpose(2, 0, 1, 3)
            w01[j] = np.stack([a, b], axis=3)
            # w2^T [DFF, D] -> [NDS, P, KF, DSW] (contiguous 8KB lines)
            w2t[j] = w2[e].T.reshape(KF, P, NDS, DSW).transpose(2, 1, 0, 3)
        # w01 k-chunks: [EPC, NFG, KD2, P, KP, 2, FGP] (contiguous lines)
        m["w01"] = np.ascontiguousarray(
            w01.reshape(EPC, NFG, KD2, KP, P, 2, FGP)
               .transpose(0, 1, 2, 4, 3, 5, 6))
        m["w2t"] = w2t
        m["scv"] = scv
        in_maps.append(m)
    return in_maps, idx, assign, tuple(TGs), (Tn, Dn, DFFn)


def _combine(results, idx, assign, shapes):
    """Unshard: scatter-add per-expert outputs into the dense [T, D] output."""
    Tn, Dn, _ = shapes
    out = np.zeros((Tn, Dn), np.float32)
    for c in range(NCORES):
        for j in range(EPC):
            e = assign[c][j]
            ids = idx[e]
            if len(ids):
                # y layout [KD, P, TG] = y^T tiles (D on partitions)
                y = results[c][f"y{j}"]
                yt = y.reshape(Dn, y.shape[-1])[:, :len(ids)]
                out[ids] += yt.T.astype(np.float32)
    return out


def _ensure_axon_ntff_hook():
    """Provide antenv.axon_hooks if the image's antenv stub lacks it.

    concourse.bass_utils imports it unconditionally when BASS_TRACE/trace is
    set under axon; without this the run crashes. When libaxon_pjrt.so exposes
    the NRT-profile symbols we also install the real hook so NTFF profiling
    (HW exec times) works; otherwise tracing degrades to a warning.
    """
    import sys
    import types
    try:
        import antenv.axon_hooks  # noqa: F401
        return
    except ImportError:
        pass
    try:
        import antenv

        mod = types.ModuleType("antenv.axon_hooks")
        _state = {"hook": None}
        mod.set_axon_ntff_profile_hook = lambda h: _state.__setitem__("hook", h)
        mod.get_axon_ntff_profile_hook = lambda: _state["hook"]
        sys.modules["antenv.axon_hooks"] = mod
        antenv.axon_hooks = mod
        try:
            from trn_agent_boot.trn_boot import _ntff_profile_via_ctypes

            so = "/opt/axon/libaxon_pjrt.so"
            if os.path.exists(so):
                mod.set_axon_ntff_profile_hook(_ntff_profile_via_ctypes(so))
        except Exception:
            pass
    except Exception:
        pass


def kernel(**inputs) -> np.ndarray:
    global LAST_RESULT
    _ensure_axon_ntff_hook()
    from concourse.bass_utils import run_bass_kernel_spmd

    in_maps, idx, assign, TGs, shapes = _prep_host(inputs)

    key = TGs + shapes
    nc = _PROGRAM_CACHE.get(key)
    if nc is None:
        nc = _build_program(list(TGs), d=shapes[1], dff=shapes[2])
        _PROGRAM_CACHE[key] = nc

    res = run_bass_kernel_spmd(nc, in_maps, core_ids=list(range(NCORES)))
    LAST_RESULT = res
    return _combine(res.results, idx, assign, shapes)


# revision 40
# speedup vs baseline: 1.0758x; 1.0385x over previous
"""DeepSeek-style MoE block (SwiGLU experts, top-k routing) on 8 Trainium2 cores.

Expert-parallel sharding: each of the 8 cores owns E/8 = 2 experts and receives
only the tokens routed to those experts (host-side dispatch). The device kernel
computes, per expert e with gathered/padded tokens XT [D, TG] (transposed):

    GT = W0e @ X^T             (PSUM, bf16 matmuls, DFF on partitions)
    UT = W1e @ X^T
    HT = silu(s0*GT) * UT      (SBUF bf16, [DFF, TG])
    Y^T = (W2e @ HT) * coef    (D on partitions, tokens on the free axis,
                                coef = s1*s2*cw broadcast to all partitions,
                                applied during the PSUM->SBUF cast)

The host then scatter-adds each expert's Y columns into the dense [T, D]
output (the unshard/combine step for expert-parallel sharding).

Perf notes (from NTFF traces; baseline fp32 kernel was 210.7 us, this one
measures ~140-146 us on hardware):
 - all matmul operands (xt, w01, w2, ht) are bf16: 1 col/cycle warm PE rate
   with half the fp32 DMA traffic, and bf16 enables the compiler's fast
   weight load (FWL) so LDWEIGHTS hides under the matmul stream (fp32
   LDWEIGHTS at ~198 ns/load nearly serialized with the MMs). PSUM
   accumulation and the activation path stay fp32.
 - Experts are sorted by routed-token count and paired big+small onto
   cores: slot 0 pads to the max big count, slot 1 to the max small count
   (ceil-8). Phase 2 streams ht (TG token columns) against stationary w2
   128x128 blocks, so BOTH phases' matmul columns scale with the actual
   routing load: ~10% fewer PE cycles than uniform 384-token padding.
 - TRN2 has two physical HWDGE rings (SP + Activation). All large DMAs
   alternate between nc.sync and nc.scalar so the two rings run in parallel.
 - DMA pacing is everything. The 16 SDMA engines round-robin over in-flight
   transfers at packet granularity, so completions are fair-shared, not
   issue-ordered: many small eager loads starve the one the PE needs first.
   w01 streams as 512 KB 4-k-tile chunks (the first one ramped 256+256 KB so
   matmul k=0 starts early); w2 chunks are issued at f-group boundaries
   fg>=1 so they spread across phase 1 without touching the cold-start
   window; the next expert's xt and fg0 w01 ride the ring FIFO behind the
   current expert's full w01 stream. Pool bufs (w01=5, w2=4) are sized so no
   ring-head tile-slot wait ever blocks a later-needed stream.
 - Split PSUM pools (6 banks phase 1 + 2 banks phase 2) so the phases never
   chain on each other's bank releases at expert/phase boundaries.
 - Remaining span: ~7 us fixed NEFF preamble + ~5 us first-data wait, then
   the fg0-fg2 window of expert 0 rides the HBM roofline (engines ~100%
   busy) with the PE HAM-cold underneath, then PE-bound warm matmuls to the
   end; ~5 us tail = last store + fixed teardown.
"""

import os
import numpy as np

T, D, DFF, E, TOPK = 1024, 2048, 1024, 16, 6
NCORES, P = 8, 128
EPC = E // NCORES  # experts per core

# Set by kernel() after each run: BassKernelResults (exec_time_ns when traced).
LAST_RESULT = None

_PROGRAM_CACHE = {}

XC = 4   # xt k-tiles per DMA chunk
KP = 4   # w01 k-tiles per DMA chunk


def _mwidths(TG):
    """Token-tile widths for phase 2 (tokens become PSUM partitions)."""
    out = []
    left = TG
    while left > 0:
        out.append(min(P, left))
        left -= P
    return out


def _build_program(TGs, d=D, dff=DFF, use_silu=True):
    """Build + compile the SPMD single-core Bass program.

    TGs: per-slot padded token counts (multiples of 8, <= 512).
    use_silu=False decomposes silu into sigmoid+mul (CoreSim lacks Silu).
    """
    import concourse.bacc as bacc
    import concourse.mybir as mybir
    import concourse.tile as tile

    f32 = mybir.dt.float32
    bf16 = mybir.dt.bfloat16
    Silu = mybir.ActivationFunctionType.Silu

    KD = d // P        # k-tiles over D (contraction of W0/W1 matmuls)
    KF = dff // P      # k-tiles over DFF (contraction of W2 matmul)
    DSW = min(512, d)  # output D slice width
    NDS = d // DSW     # output D slices
    FG = 2             # DFF f-tiles per PSUM group (2G + 2U = 4 banks/group)
    FGP = FG * P
    NFG = KF // FG
    NXC = KD // XC
    KD2 = KD // KP
    NTmax = max(len(_mwidths(TG)) for TG in TGs)

    assert all(TG <= 512 for TG in TGs)

    nc = bacc.Bacc("TRN2", target_bir_lowering=False, debug=False)

    xt_ds = [nc.dram_tensor(f"xt{e}", [NXC, P, XC, TGs[e]], bf16,
                            kind="ExternalInput").ap() for e in range(EPC)]
    w01_d = nc.dram_tensor("w01", [EPC, NFG, KD2, P, KP, 2, FGP], bf16,
                           kind="ExternalInput").ap()
    w2t_d = nc.dram_tensor("w2t", [EPC, NDS, P, KF, DSW], bf16,
                           kind="ExternalInput").ap()
    sc_d = nc.dram_tensor("scv", [EPC, P, 1 + NTmax], f32,
                          kind="ExternalInput").ap()
    # coef replicated across partitions: phase 2 scales PSUM columns (tokens)
    # during the PSUM->SBUF cast, so the per-token combine weight needs to be
    # available on every output partition.
    cp_ds = [nc.dram_tensor(f"coefp{e}", [P, TGs[e]], f32,
                            kind="ExternalInput").ap() for e in range(EPC)]
    # y layout [KD, P, TG]: phase-2 output has D on partitions, tokens free
    y_ds = [nc.dram_tensor(f"y{e}", [KD, P, TGs[e]], bf16,
                           kind="ExternalOutput").ap() for e in range(EPC)]

    with tile.TileContext(nc) as tc:
        # Alternate large DMAs across the two physical HWDGE rings.
        rings = [nc.sync, nc.scalar]
        ring_state = [0]

        def ring():
            ring_state[0] ^= 1
            return rings[ring_state[0]]

        with (
            tc.tile_pool(name="xt", bufs=1) as xt_pool,
            tc.tile_pool(name="w01", bufs=5) as w01_pool,
            tc.tile_pool(name="w2", bufs=4) as w2_pool,
            tc.tile_pool(name="ht", bufs=1) as ht_pool,
            tc.tile_pool(name="act", bufs=6) as act_pool,
            tc.tile_pool(name="out", bufs=8) as out_pool,
            tc.tile_pool(name="sc", bufs=2) as sc_pool,
            tc.tile_pool(name="pgu", bufs=6, space="PSUM") as pgu_pool,
            tc.tile_pool(name="py", bufs=2, space="PSUM") as py_pool,
        ):
            # Tiles created by the previous expert's prefetch block (the ring
            # FIFO paces their descriptor-gen behind the in-order w01 stream,
            # so the prefetch costs no early bandwidth).
            xts = {}        # e -> xt tile (per-slot tag: bufs=1 suffices)
            pre01 = {}      # (e, kb) -> fg0 w01 chunk tile

            def get_xt(e):
                if e not in xts:
                    xts[e] = xt_pool.tile([P, KD, TGs[e]], bf16, tag=f"xt{e}",
                                          name=f"xt_{e}")
                return xts[e]

            for e in range(EPC):
                TG = TGs[e]

                # --- inputs for this expert ---
                xt = get_xt(e)
                scv = sc_pool.tile([P, 1 + NTmax], f32, tag="scv")
                nc.gpsimd.dma_start(scv[:], sc_d[e])
                s0_sb = scv[:, 0:1]
                # coefp rides the ring at the fg1 boundary (below): it is
                # only needed in phase 2, so it must not steal early HBM
                # bandwidth from the critical fg0 feed.
                coefp = sc_pool.tile([P, TG], f32, tag=f"cp{e}",
                                     name=f"coefp_{e}")

                # --- phase 1: HT = silu(s0 * W0 xT) * (W1 xT), [DFF, TG] ---
                # w2 chunks are issued at f-group boundaries (fg>=1): the ring
                # FIFO paces them behind that f-group's w01 stream, spreading
                # the ~4 MB w2 prefetch across phase 1 instead of cramming it
                # into the (now short) phase-2 window — while keeping it out
                # of the cold-start fg0 window.
                w2bs = []

                def issue_w2():
                    dsi = len(w2bs)
                    w2b = w2_pool.tile([P, KF, DSW], bf16, tag="w2b",
                                       name=f"w2b_{e}_{dsi}")
                    ring().dma_start(w2b[:], w2t_d[e, dsi])
                    w2bs.append(w2b)

                ht = ht_pool.tile([P, KF, TG], bf16, tag=f"ht{e}")
                for fg in range(NFG):
                    if fg == 1:
                        ring().dma_start(coefp[:], cp_ds[e])
                    if fg >= 1 and len(w2bs) < NDS:
                        issue_w2()
                    psG = [pgu_pool.tile([P, 512], f32, tag="pgu",
                                         name=f"psG_{e}_{fg}_{j}")
                           for j in range(FG)]
                    psU = [pgu_pool.tile([P, 512], f32, tag="pgu",
                                         name=f"psU_{e}_{fg}_{j}")
                           for j in range(FG)]
                    for kb in range(KD2):
                        if e == 0 and fg == 0:
                            # interleave x chunk loads with the first
                            # f-group's weight loads so matmul k can start
                            # as soon as its own xt/w01 slices land; the
                            # first chunk is halved so matmul k=0 waits on
                            # ~180 KB of x instead of ~360 KB
                            for c in range(kb * NXC // KD2,
                                           (kb + 1) * NXC // KD2):
                                if c == 0:
                                    h = XC // 2
                                    ring().dma_start(
                                        xt[:, :h, :], xt_ds[e][0][:, :h])
                                    ring().dma_start(
                                        xt[:, h:XC, :], xt_ds[e][0][:, h:])
                                else:
                                    ring().dma_start(
                                        xt[:, c * XC:(c + 1) * XC, :],
                                        xt_ds[e][c])
                        if (e, kb) in pre01 and fg == 0:
                            w01b = pre01.pop((e, kb))
                        else:
                            w01b = w01_pool.tile([P, KP, 2, FGP], bf16,
                                                 tag="w01b")
                            if e == 0 and fg == 0 and kb == 0:
                                # halve the very first weight DMA: the first
                                # matmul waits on ~256 KB instead of 512 KB
                                h = KP // 2
                                ring().dma_start(w01b[:, :h],
                                                 w01_d[e, fg, kb][:, :h])
                                ring().dma_start(w01b[:, h:],
                                                 w01_d[e, fg, kb][:, h:])
                            else:
                                ring().dma_start(w01b[:], w01_d[e, fg, kb])
                        for kk in range(KP):
                            k = kb * KP + kk
                            for j in range(FG):
                                nc.tensor.matmul(
                                    psG[j][:, :TG],
                                    w01b[:, kk, 0, j * P:(j + 1) * P],
                                    xt[:, k, :],
                                    start=(k == 0), stop=(k == KD - 1))
                                nc.tensor.matmul(
                                    psU[j][:, :TG],
                                    w01b[:, kk, 1, j * P:(j + 1) * P],
                                    xt[:, k, :],
                                    start=(k == 0), stop=(k == KD - 1))
                    for j in range(FG):
                        f = fg * FG + j
                        sig = act_pool.tile([P, 512], f32, tag="sig")
                        ht_f = ht[:, f, :]
                        if use_silu:
                            nc.scalar.activation(
                                sig[:, :TG], psG[j][:, :TG], Silu,
                                scale=s0_sb)
                            nc.vector.tensor_mul(
                                ht_f, sig[:, :TG], psU[j][:, :TG])
                        else:
                            nc.scalar.activation(
                                sig[:, :TG], psG[j][:, :TG],
                                mybir.ActivationFunctionType.Sigmoid,
                                scale=s0_sb)
                            nc.vector.tensor_mul(
                                ht_f, sig[:, :TG], psU[j][:, :TG])
                            nc.vector.tensor_mul(
                                ht_f, ht_f, psG[j][:, :TG])

                # --- post-phase-1 prefetch block (ring FIFO = these gen only
                # after all of this expert's w01 chunk D2Ds, i.e. paced to
                # late phase 1 — no bandwidth stolen from the phase-1 feed).
                # Order: this expert's remaining w2 chunks, then the next
                # expert's xt and fg0 w01 chunks (needed at the next phase-1
                # start, before this expert's y-outs unblock the rings).
                while len(w2bs) < NDS:
                    issue_w2()
                if e + 1 < EPC:
                    nxt = get_xt(e + 1)
                    for c in range(NXC):
                        ring().dma_start(nxt[:, c * XC:(c + 1) * XC, :],
                                         xt_ds[e + 1][c])
                    for kb in range(KD2):
                        w01b = w01_pool.tile([P, KP, 2, FGP], bf16,
                                             tag="w01b")
                        ring().dma_start(w01b[:], w01_d[e + 1, 0, kb])
                        pre01[(e + 1, kb)] = w01b

                # --- phase 2: Y^T = W2 @ HT, scaled per token (free axis) ---
                # w2 128x128 blocks are stationary and ht streams as the
                # moving operand, so the matmul columns scale with the padded
                # token count TG (<=512) instead of the fixed D-slice width.
                for dsi in range(NDS):
                    w2b = w2bs[dsi]
                    for j in range(DSW // P):
                        i = dsi * (DSW // P) + j
                        last = (e == EPC - 1 and dsi == NDS - 1
                                and j == DSW // P - 1)
                        # the very last tile computes in two column halves so
                        # the final scale+store overlaps the final matmuls
                        splits = ([(0, TG // 2 // 8 * 8),
                                   (TG // 2 // 8 * 8, TG)] if last
                                  else [(0, TG)])
                        psY = py_pool.tile([P, 512], f32, tag="py",
                                           name=f"psY_{e}_{i}")
                        ysb = out_pool.tile([P, 512], bf16, tag="ysb")
                        for (ta, tb) in splits:
                            for k in range(KF):
                                nc.tensor.matmul(
                                    psY[:, ta:tb],
                                    w2b[:, k, j * P:(j + 1) * P],
                                    ht[:, k, ta:tb],
                                    start=(k == 0), stop=(k == KF - 1))
                            nc.vector.tensor_mul(
                                ysb[:, ta:tb], psY[:, ta:tb],
                                coefp[:, ta:tb])
                            ring().dma_start(y_ds[e][i][:, ta:tb],
                                             ysb[:, ta:tb])

    nc.compile()
    return nc


def _prep_host(inputs):
    """Host-side dispatch: routing weights, expert->core/slot assignment,
    per-expert token gather, device layouts."""
    import ml_dtypes
    bf16 = ml_dtypes.bfloat16

    x = np.ascontiguousarray(np.asarray(inputs["x"], dtype=np.float32))
    w0 = np.asarray(inputs["w0"], dtype=np.float32)
    w1 = np.asarray(inputs["w1"], dtype=np.float32)
    w2 = np.asarray(inputs["w2"], dtype=np.float32)
    s0 = np.asarray(inputs["s0"], dtype=np.float32)
    s1 = np.asarray(inputs["s1"], dtype=np.float32)
    s2 = np.asarray(inputs["s2"], dtype=np.float32)
    se = np.asarray(inputs["selected_experts"]).astype(np.int64)
    rw = np.asarray(inputs["routing_weights"], dtype=np.float32)

    Tn, Dn = x.shape
    En, DFFn, _ = w0.shape
    KD = Dn // P
    KF = DFFn // P
    KD2 = KD // KP
    NXC = KD // XC
    DSW = min(512, Dn)
    NDS = Dn // DSW
    FG = 2
    FGP = FG * P
    NFG = KF // FG

    # combine weight per (expert, token): sum of routing weights over top-k
    cw = np.zeros((En, Tn), np.float32)
    cols = np.arange(Tn)
    for k in range(se.shape[1]):
        np.add.at(cw, (se[:, k], cols), rw[:, k])

    idx = [np.flatnonzero(cw[e] != 0.0) for e in range(En)]
    counts = np.array([len(i) for i in idx])

    # Sort experts by token count; slot s of core c gets the (s*NCORES+c)-th
    # largest. Per-slot padded width = ceil8(max count in that slot), so the
    # matmul columns track the actual routing load instead of worst-case.
    order = np.argsort(-counts, kind="stable")
    assign = [[int(order[s * NCORES + c]) for s in range(EPC)]
              for c in range(NCORES)]
    TGs = []
    for s in range(EPC):
        mx = max(counts[order[s * NCORES + c]] for c in range(NCORES))
        TGs.append(max(256, int(-(-mx // 8) * 8)))
    NTmax = max(len(_mwidths(TG)) for TG in TGs)

    xT = np.ascontiguousarray(x.T).astype(bf16)  # [D, T]
    in_maps = []
    for c in range(NCORES):
        m = {}
        scv = np.zeros((EPC, P, 1 + NTmax), np.float32)
        w01 = np.empty((EPC, NFG, KD, P, 2, FGP), bf16)
        w2t = np.empty((EPC, NDS, P, KF, DSW), bf16)
        for j in range(EPC):
            e = assign[c][j]
            ids = idx[e]
            TG = TGs[j]
            xt = np.zeros((Dn, TG), bf16)
            xt[:, :len(ids)] = xT[:, ids]
            # xt chunks: [NXC, P, XC, TG] with contiguous partition lines
            m[f"xt{j}"] = np.ascontiguousarray(
                xt.reshape(NXC, XC, P, TG).transpose(0, 2, 1, 3))
            coef = np.zeros(TG, np.float32)
            coef[:len(ids)] = s1[e] * s2[e] * cw[e, ids]
            m[f"coefp{j}"] = np.ascontiguousarray(
                np.broadcast_to(coef, (P, TG)))
            scv[j, :, 0] = s0[e]
            # [D, DFF] -> [NFG, KD, P, FGP] blocks, w0/w1 interleaved
            a = w0[e].T.reshape(KD, P, NFG, FGP).transpose(2, 0, 1, 3)
            b = w1[e].T.reshape(KD, P, NFG, FGP).transpose(2, 0, 1, 3)
            w01[j] = np.stack([a, b], axis=3)
            # w2^T [DFF, D] -> [NDS, P, KF, DSW] (contiguous 8KB lines)
            w2t[j] = w2[e].T.reshape(KF, P, NDS, DSW).transpose(2, 1, 0, 3)
        # w01 k-chunks: [EPC, NFG, KD2, P, KP, 2, FGP] (contiguous lines)
        m["w01"] = np.ascontiguousarray(
            w01.reshape(EPC, NFG, KD2, KP, P, 2, FGP)
               .transpose(0, 1, 2, 4, 3, 5, 6))
        m["w2t"] = w2t
        m["scv"] = scv
        in_maps.append(m)
    return in_maps, idx, assign, tuple(TGs), (Tn, Dn, DFFn)


def _combine(results, idx, assign, shapes):
    """Unshard: scatter-add per-expert outputs into the dense [T, D] output."""
    Tn, Dn, _ = shapes
    out = np.zeros((Tn, Dn), np.float32)
    for c in range(NCORES):
        for j in range(EPC):
            e = assign[c][j]
            ids = idx[e]
            if len(ids):
                # y layout [KD, P, TG] = y^T tiles (D on partitions)
                y = results[c][f"y{j}"]
                yt = y.reshape(Dn, y.shape[-1])[:, :len(ids)]
                out[ids] += yt.T.astype(np.float32)
    return out


def _ensure_axon_ntff_hook():
    """Provide antenv.axon_hooks if the image's antenv stub lacks it.

    concourse.bass_utils imports it unconditionally when BASS_TRACE/trace is
    set under axon; without this the run crashes. When libaxon_pjrt.so exposes
    the NRT-profile symbols we also install the real hook so NTFF profiling
    (HW exec times) works; otherwise tracing degrades to a warning.
    """
    import sys
    import types
    try:
        import antenv.axon_hooks  # noqa: F401
        return
    except ImportError:
        pass
    try:
        import antenv

        mod = types.ModuleType("antenv.axon_hooks")
        _state = {"hook": None}
        mod.set_axon_ntff_profile_hook = lambda h: _state.__setitem__("hook", h)
        mod.get_axon_ntff_profile_hook = lambda: _state["hook"]
        sys.modules["antenv.axon_hooks"] = mod
        antenv.axon_hooks = mod
        try:
            from trn_agent_boot.trn_boot import _ntff_profile_via_ctypes

            so = "/opt/axon/libaxon_pjrt.so"
            if os.path.exists(so):
                mod.set_axon_ntff_profile_hook(_ntff_profile_via_ctypes(so))
        except Exception:
            pass
    except Exception:
        pass


def kernel(**inputs) -> np.ndarray:
    global LAST_RESULT
    _ensure_axon_ntff_hook()
    from concourse.bass_utils import run_bass_kernel_spmd

    in_maps, idx, assign, TGs, shapes = _prep_host(inputs)

    key = TGs + shapes
    nc = _PROGRAM_CACHE.get(key)
    if nc is None:
        nc = _build_program(list(TGs), d=shapes[1], dff=shapes[2])
        _PROGRAM_CACHE[key] = nc

    res = run_bass_kernel_spmd(nc, in_maps, core_ids=list(range(NCORES)))
    LAST_RESULT = res
    return _combine(res.results, idx, assign, shapes)
